# revision 27
# baseline (speedup 1.0000x reference)
"""GAT-style edge-affinity layer (nn_Decode_Cora) on 8 Trainium2 NeuronCores.

Sharding: each core owns a 512-node slice of the graph's SOURCE nodes j. It
computes attention-numerator/denominator partial sums over its 512 j for ALL
4096 destinations i, and a two-stage ReduceScatter (destination-major layout)
hands each core its 512 output rows for the final divide + ELU.

Math: softmax rows are invariant to per-row scaling, and per-j scalings can be
folded into the aggregation weights, so with
    esl_i = exp(0.8*sl_i), r_j = exp(-0.8*sr_j), esr1_j = exp(sr_j):
    p[i,j] = mask * exp(lrelu(sl_i + sr_j)) ∝ esr1_j * mask * max(esl_i, r_j)
No per-element exp: the inner loop is one 4x-mode tensor_scalar max (or a
2-pass relu+add on the otherwise-idle ACT engine) and one 2x-mode
tensor_tensor mask multiply; esr1_j scales the matmul weight columns
(g̃ = g*esr1, denominator column = esr1).

sl_i is needed for ALL i on every core; instead of an AllGather (which eats
~30us of inter-core skew + serialization before the main loop can start),
every core gets the full vert^T (f16) and a host-precomputed wa = W @ (0.8
a_l) [F,H], and projects sl = wa^T @ vert^T itself, chunked so it overlaps
the vert DMA stream.
"""

import sys

for _p in ("/opt/trn_rl_repo",):
    if _p not in sys.path:
        sys.path.append(_p)

import numpy as np
import ml_dtypes

import concourse.bass as bass
import concourse.bacc as bacc
import concourse.mybir as mybir
import concourse.tile as tile
from concourse.masks import make_identity

f32 = mybir.dt.float32
f16 = mybir.dt.float16

N = 4096          # nodes
F = 1433          # input features
FP = 1536         # padded features (12 * 128)
KT = FP // 128    # 12 contraction tiles
H = 8             # heads
DH = 8            # per-head dim
HD = H * DH       # 64
NC = 8            # cores
NL = N // NC      # 512 nodes per core
NCH = NL // 128   # 4 local j-chunks
NIS = N // 512    # 8 destination column slices
LRELU = 0.2

# (head, chunk) pairs whose max runs as relu+add on the ACT engine instead of
# one DVE tensor_scalar, to balance the two engines in the main loop.
ACT_MAX = {(h, 2) for h in range(H)} | {(h, 3) for h in range(0, H, 2)}

_STATE = {}

EXP = mybir.ActivationFunctionType.Exp
RELU = mybir.ActivationFunctionType.Relu
IDENT = mybir.ActivationFunctionType.Identity
COPY = mybir.ActivationFunctionType.Copy


def _finish_quarter(nc, sp, numt_rsQ, out, q):
    """Divide + ELU + output store for head pair (2q, 2q+1)."""
    for b in range(NL // 128):
        nf = sp.tile([128, 18], f32, name="nf")
        nc.sync.dma_start(nf[:], numt_rsQ[q][128 * b:128 * (b + 1), :])
        nfr = nf.rearrange("p (h k) -> p h k", k=9)
        rec = sp.tile([128, 2], f32, name="rec")
        nc.vector.reciprocal(rec[:], nfr[:, :, 8])
        aout = sp.tile([128, 16], f32, name="aout")
        for hh in range(2):
            nc.vector.tensor_scalar(aout[:, 8 * hh:8 * (hh + 1)],
                                    nfr[:, hh, 0:8],
                                    rec[:, hh:hh + 1], None,
                                    mybir.AluOpType.mult)
        # elu(x) = relu(x) - 1 + exp(min(x, 0))
        xm = sp.tile([128, 16], f32, name="xm")
        nc.vector.tensor_scalar(xm[:], aout[:], 0.0, None, mybir.AluOpType.min)
        ex = sp.tile([128, 16], f32, name="ex")
        nc.scalar.activation(ex[:], xm[:], EXP)
        r1 = sp.tile([128, 16], f32, name="r1")
        nc.vector.tensor_scalar(r1[:], aout[:], 0.0, -1.0,
                                mybir.AluOpType.max, mybir.AluOpType.add)
        ot = sp.tile([128, 16], f32, name="ot")
        nc.vector.tensor_tensor(ot[:], ex[:], r1[:], mybir.AluOpType.add)
        nc.sync.dma_start(
            out[128 * b:128 * (b + 1), 16 * q:16 * (q + 1)], ot[:])


def _build_program(repeat=1, null=False, nocc=False, debug=False, variant='b'):
    nc = bacc.Bacc("TRN2", target_bir_lowering=False, debug=False, num_devices=NC)

    vt = nc.dram_tensor("vt", [FP, NL], f16, kind="ExternalInput")
    vtf = nc.dram_tensor("vtf", [FP, N], f16, kind="ExternalInput")
    wp = nc.dram_tensor("wp", [FP, HD], f16, kind="ExternalInput")
    wa = nc.dram_tensor("wa", [FP, H], f16, kind="ExternalInput")
    ar = nc.dram_tensor("ar", [128, H], f32, kind="ExternalInput")
    mskt = nc.dram_tensor("mskt", [NL, N], f16, kind="ExternalInput")
    out = nc.dram_tensor("out", [NL, HD], f32, kind="ExternalOutput")

    # DRAM staging for the esl broadcast (SBUF sources can't be partition-
    # broadcast) + the four-stage ReduceScatter (head pairs)
    esl_dram = nc.dram_tensor("esl_dram", [H, N], f16)
    numtQ = [nc.dram_tensor(f"numtQ{q}", [N, 18], f32) for q in range(4)]
    numt_rsQ = [nc.dram_tensor(f"numt_rsQ{q}", [NL, 18], f32) for q in range(4)]

    if null:
        with tile.TileContext(nc) as tc:
            with tc.tile_pool(name="np0", bufs=1) as p0:
                t0 = p0.tile([128, 64], f16)
                t1 = p0.tile([128, 64], f32)
                for b in range(NL // 128):
                    nc.sync.dma_start(t0[:], vt[128 * b:128 * (b + 1), 0:64])
                    nc.vector.tensor_copy(t1[:], t0[:])
                    nc.sync.dma_start(out[128 * b:128 * (b + 1), :], t1[:])
        nc.compile()
        return nc

    with tile.TileContext(nc) as tc:
        with (
            tc.tile_pool(name="const", bufs=1) as cp,
            tc.tile_pool(name="psum", bufs=8, space="PSUM") as pp,
        ):
            # ---- resident tiles ----
            w_sb = cp.tile([128, KT, HD], f16)
            nc.sync.dma_start(w_sb[:], wp[:].rearrange("(k p) d -> p k d", p=128))
            wa_sb = cp.tile([128, KT, H], f16)
            nc.sync.dma_start(wa_sb[:], wa[:].rearrange("(k p) d -> p k d", p=128))
            ar_sb = cp.tile([128, H], f32)
            nc.sync.dma_start(ar_sb[:], ar[:])
            msk_sb = cp.tile([128, NCH, N], f16)
            eslb_sb = cp.tile([128, H, N], f16)  # exp(0.8*sl_i) bcast, resident
            esl_sb = cp.tile([H, N], f16)        # exp(0.8*sl_i) row-major
            gt_sb = cp.tile([128, NL], f32)      # g^T padded to 128 partitions
            esr1_sb = cp.tile([128, NCH * H], f32)  # exp(sr), col 8c+h
            r_sb = cp.tile([128, NCH * H], f32)     # exp(-0.8 sr)
            nr_sb = cp.tile([128, NCH * H], f32)    # -exp(-0.8 sr) (ACT relu bias)
            gr_sb = cp.tile([128, NCH * 72], f16)  # lhsT per chunk/head + esr1 col
            ntb = cp.tile([128, N // 128, 72], f32, name="ntb")

            # ---- phase 1a: local projection (vt slice, single-pass f16) ----
            with tc.tile_pool(name="vtp", bufs=1) as vtp:
                vts_sb = vtp.tile([128, KT, NL], f16, name="vts")
                nc.sync.dma_start(vts_sb[:],
                                  vt[:].rearrange("(k p) n -> p k n", p=128))
                vts = [vts_sb[:, k, :] for k in range(KT)]
                # mask chunk 0 early (first tt needs it right at phase-3 start)
                nc.sync.dma_start(
                    msk_sb[:, 0, :],
                    mskt[:].rearrange("(c p) i -> c p i", p=128)[0])

                nc.vector.memset(gt_sb[64:128, :], 0.0)
                gt_ps = pp.tile([128, 512], f32, tag="bank", name="gt_ps")
                for k in range(KT):
                    nc.tensor.matmul(gt_ps[0:HD, :], w_sb[:, k, :], vts[k],
                                     start=(k == 0), stop=(k == KT - 1))
                nc.vector.tensor_copy(gt_sb[0:HD, :], gt_ps[0:HD, :])

                for c in range(NCH):
                    # g for this chunk: [128 nodes, 64]
                    g_ps = pp.tile([128, 512], f32, tag="bank", name="g_ps")
                    for k in range(KT):
                        nc.tensor.matmul(g_ps[:, 0:HD], vts[k][:, 128 * c:128 * (c + 1)],
                                         w_sb[:, k, :], start=(k == 0),
                                         stop=(k == KT - 1))
                    # sr for this chunk
                    sr_ps = pp.tile([128, 512], f32, tag="bank", name="sr_ps")
                    nc.tensor.matmul(sr_ps[:, 0:H], gt_sb[:, 128 * c:128 * (c + 1)],
                                     ar_sb[:], start=True, stop=True)
                    nc.scalar.activation(esr1_sb[:, H * c:H * (c + 1)], sr_ps[:, 0:H],
                                         EXP)
                    nc.scalar.activation(r_sb[:, H * c:H * (c + 1)], sr_ps[:, 0:H],
                                         EXP, scale=-0.8)
                    nc.vector.tensor_scalar(nr_sb[:, H * c:H * (c + 1)],
                                            r_sb[:, H * c:H * (c + 1)], -1.0, None,
                                            mybir.AluOpType.mult)
                    # lhsT tile: [g_h*esr1 | esr1] interleaved, 9 cols per head
                    grc = gr_sb[:, 72 * c:72 * (c + 1)].rearrange("p (h k) -> p h k", k=9)
                    for h in range(H):
                        nc.vector.tensor_scalar(grc[:, h, 0:8], g_ps[:, 8 * h:8 * (h + 1)],
                                                esr1_sb[:, H * c + h:H * c + h + 1],
                                                None, mybir.AluOpType.mult)
                    nc.vector.tensor_copy(grc[:, :, 8], esr1_sb[:, H * c:H * (c + 1)])

                # ---- phase 1b: sl for ALL nodes from the full vert^T stream ----
                # 8 psum banks hold [8, 512] slices; vtf tiles stream through.
                sl_banks = [pp.tile([128, 512], f32, tag="bank", name=f"slb{s}")
                            for s in range(NIS)]
                with tc.tile_pool(name="vtfp", bufs=3) as vtfp:
                    for k in range(KT):
                        vft = vtfp.tile([128, N], f16, name="vtf")
                        nc.sync.dma_start(
                            vft[:], vtf[:].rearrange("(k p) n -> k p n", k=KT)[k])
                        for s in range(NIS):
                            nc.tensor.matmul(sl_banks[s][0:H, :], wa_sb[:, k, :],
                                             vft[:, 512 * s:512 * (s + 1)],
                                             start=(k == 0), stop=(k == KT - 1))
                for s in range(NIS):
                    nc.scalar.activation(esl_sb[:, 512 * s:512 * (s + 1)],
                                         sl_banks[s][0:H, :], EXP)
                nc.sync.dma_start(esl_dram[:], esl_sb[:])

            # ---- phase 2: broadcast esl rows to 128 partitions (via DRAM) ----
            # head 0 first (gates phase 3), then mask chunks 1-3, then the rest.
            def _bcast(h):
                _src = esl_dram[h:h + 1, :].rearrange("o (s f) -> o s f", s=NIS)
                nc.sync.dma_start(
                    eslb_sb[:, h, :].rearrange("p (s f) -> p s f", s=NIS),
                    _src.to_broadcast([128, NIS, NL]),
                )
            _bcast(0)
            nc.sync.dma_start(
                msk_sb[:, 1:NCH, :],
                mskt[:].rearrange("(c p) i -> p c i", p=128)[:, 1:NCH, :])
            for h in range(1, H):
                _bcast(h)

            # ---- phase 3: main attention loop ----
            with (
                tc.tile_pool(name="tp", bufs=3) as tp,
                tc.tile_pool(name="urp", bufs=2) as urp,
                tc.tile_pool(name="pmp", bufs=4) as pmp,
                tc.tile_pool(name="small", bufs=4) as sp,
            ):
              for _rep in range(repeat):
                for h in range(H):
                    slb = eslb_sb[:, h, :]
                    # one PSUM bank holds the whole head: [128 dest, 32 blk * 9]
                    bank = pp.tile([128, 512], f32, tag="bank", name="bank")
                    for c in range(NCH):
                        u = tp.tile([128, N], f16, name="umax")
                        if (h, c) in ACT_MAX:
                            # max(esl, r) = r + relu(esl - r), on ACT
                            u1 = urp.tile([128, N], f16, name="urelu")
                            nc.scalar.activation(u1[:], slb,
                                                 RELU,
                                                 bias=nr_sb[:, H * c + h:H * c + h + 1])
                            nc.scalar.activation(u[:], u1[:],
                                                 IDENT,
                                                 bias=r_sb[:, H * c + h:H * c + h + 1])
                        else:
                            nc.vector.tensor_scalar(u[:], slb,
                                                    r_sb[:, H * c + h:H * c + h + 1],
                                                    None, mybir.AluOpType.max)
                        pm = pmp.tile([128, N], f16, name="pm")
                        nc.vector.tensor_tensor(pm[:], u[:], msk_sb[:, c, :],
                                                mybir.AluOpType.mult)
                        rhs = gr_sb[:, 72 * c + 9 * h:72 * c + 9 * (h + 1)]
                        for ib in range(N // 128):
                            nc.tensor.matmul(bank[:, 9 * ib:9 * (ib + 1)],
                                             pm[:, 128 * ib:128 * (ib + 1)], rhs,
                                             start=(c == 0 and ib == 0),
                                             stop=(c == NCH - 1 and ib == N // 128 - 1),
                                             skip_group_check=True)
                    # drain the head's bank on the ACT engine (DVE stays hot)
                    nc.scalar.activation(
                        ntb[:, :, 9 * h:9 * (h + 1)],
                        bank[:, 0:288].rearrange("p (b k) -> p b k", k=9), COPY)

                    # quarter q = heads (2q, 2q+1): kick its ReduceScatter as
                    # soon as both heads drained; run its divide+ELU two heads
                    # later (so the in-order DVE queue never waits on the RS),
                    # except the last quarter which runs immediately.
                    if h % 2 == 1:
                        q = h // 2
                        nc.sync.dma_start(
                            numtQ[q][:].rearrange("(b p) m -> p b m", p=128),
                            ntb[:, :, 18 * q:18 * (q + 1)])
                        if nocc:
                            nc.sync.dma_start(numt_rsQ[q][:], numtQ[q][0:NL, :])
                        else:
                            nc.gpsimd.collective_compute(
                                "ReduceScatter", mybir.AluOpType.add,
                                replica_groups=[list(range(NC))],
                                ins=[numtQ[q][:].opt()],
                                outs=[numt_rsQ[q][:].opt()],
                            )
                        if q >= 1:
                            _finish_quarter(nc, sp, numt_rsQ, out, q - 1)
                        if q == 3:
                            _finish_quarter(nc, sp, numt_rsQ, out, 3)

    nc.compile()
    return nc


def _prep_inputs(vert, edge, W, a_l, a_r):
    vert = np.asarray(vert, dtype=np.float32)
    edge = np.asarray(edge)
    W = np.asarray(W, dtype=np.float32)
    a_l = np.asarray(a_l, dtype=np.float32)
    a_r = np.asarray(a_r, dtype=np.float32)

    vtp32 = np.zeros((FP, N), dtype=np.float32)
    vtp32[:F] = vert.T
    vtp = vtp32.astype(np.float16)
    wp32 = np.zeros((FP, HD), dtype=np.float32)
    wp32[:F] = W
    wp = wp32.astype(np.float16)

    # wa[f, h] = sum_d W[f, (h,d)] * 0.8 * a_l[h, d]
    wa = np.einsum('fhd,hd->fh', wp32.reshape(FP, H, DH), 0.8 * a_l)
    wa = wa.astype(np.float16)

    ar8 = np.zeros((128, H), dtype=np.float32)
    for h in range(H):
        ar8[8 * h:8 * (h + 1), h] = a_r[h]

    maskT = (edge != 0).astype(np.float16)  # [i, j] -> transpose below

    in_maps = []
    for c in range(NC):
        sl = slice(512 * c, 512 * (c + 1))
        in_maps.append({
            "vt": np.ascontiguousarray(vtp[:, sl]),
            "vtf": vtp,
            "wp": wp,
            "wa": wa,
            "ar": ar8,
            "mskt": np.ascontiguousarray(maskT[:, sl].T),
        })
    return in_maps


def _get_runner(repeat=1, null=False, variant='b'):
    """Build (once) and return a callable in_maps -> list of per-core outputs."""
    key = f"runner{repeat}_{null}_{variant}"
    if key in _STATE:
        return _STATE[key]

    nc = _build_program(repeat, null, variant=variant)
    _STATE[f"program{repeat}_{null}_{variant}"] = nc

    import jax
    from jax.sharding import Mesh, PartitionSpec
    from jax.experimental.shard_map import shard_map
    from concourse import bass2jax
    from concourse.bass2jax import _bass_exec_p, partition_id_tensor

    bass2jax.install_neuronx_cc_hook()

    partition_name = nc.partition_id_tensor.name if nc.partition_id_tensor else None
    in_names, out_names, out_avals, zero_shapes = [], [], [], []
    for alloc in nc.m.functions[0].allocations:
        if not isinstance(alloc, mybir.MemoryLocationSet):
            continue
        name = alloc.memorylocations[0].name
        if alloc.kind == "ExternalInput":
            if name != partition_name:
                in_names.append(name)
        elif alloc.kind == "ExternalOutput":
            shape = tuple(alloc.tensor_shape)
            dtype = mybir.dt.np(alloc.dtype)
            out_names.append(name)
            out_avals.append(jax.core.ShapedArray(shape, dtype))
            zero_shapes.append((shape, dtype))
    n_params = len(in_names)
    n_outs = len(out_avals)
    all_in_names = list(in_names) + list(out_names)
    if partition_name is not None:
        all_in_names.append(partition_name)
    donate = tuple(range(n_params, n_params + n_outs))

    def _body(*args):
        operands = list(args)
        if partition_name is not None:
            operands.append(partition_id_tensor())
        outs = _bass_exec_p.bind(
            *operands,
            out_avals=tuple(out_avals),
            in_names=tuple(all_in_names),
            out_names=tuple(out_names),
            lowering_input_output_aliases=(),
            sim_require_finite=True,
            sim_require_nnan=True,
            nc=nc,
        )
        return tuple(outs)

    devices = jax.devices()[:NC]
    mesh = Mesh(np.asarray(devices), ("core",))
    in_specs = (PartitionSpec("core"),) * (n_params + n_outs)
    out_specs = (PartitionSpec("core"),) * n_outs
    sharded = jax.jit(
        shard_map(_body, mesh=mesh, in_specs=in_specs, out_specs=out_specs,
                  check_rep=False),
        donate_argnums=donate, keep_unused=True,
    )

    def runner(in_maps):
        concat_in = [
            np.concatenate([np.asarray(in_maps[c][nm]) for c in range(NC)], axis=0)
            for nm in in_names
        ]
        concat_zeros = [
            np.zeros((NC * s[0], *s[1:]), dt) for (s, dt) in zero_shapes
        ]
        out_arrs = sharded(*concat_in, *concat_zeros)
        out_arrs = [np.asarray(a) for a in out_arrs]
        return [
            {nm: out_arrs[i].reshape(NC, *out_avals[i].shape)[c]
             for i, nm in enumerate(out_names)}
            for c in range(NC)
        ]

    _STATE[key] = runner
    _STATE[f"internals{repeat}_{null}_{variant}"] = {
        "sharded": sharded, "in_names": in_names, "zero_shapes": zero_shapes,
        "mesh": mesh, "out_names": out_names, "out_avals": out_avals,
    }
    return runner


def kernel(vert, edge, W, a_l, a_r):
    in_maps = _prep_inputs(vert, edge, W, a_l, a_r)
    runner = _get_runner()
    results = runner(in_maps)
    return np.concatenate([results[c]["out"] for c in range(NC)], axis=0)


# revision 32
# speedup vs baseline: 1.1502x; 1.1502x over previous
"""GAT-style edge-affinity layer (nn_Decode_Cora) on 8 Trainium2 NeuronCores.

Sharding: each core owns a 512-node slice of the graph's SOURCE nodes j. It
computes attention-numerator/denominator partial sums over its 512 j for ALL
4096 destinations i, and a two-stage ReduceScatter (destination-major layout)
hands each core its 512 output rows for the final divide + ELU.

Math: softmax rows are invariant to per-row scaling, and per-j scalings can be
folded into the aggregation weights, so with
    esl_i = exp(0.8*sl_i), r_j = exp(-0.8*sr_j), esr1_j = exp(sr_j):
    p[i,j] = mask * exp(lrelu(sl_i + sr_j)) ∝ esr1_j * mask * max(esl_i, r_j)
No per-element exp: the inner loop is one 4x-mode tensor_scalar max (or a
2-pass relu+add on the otherwise-idle ACT engine) and one 2x-mode
tensor_tensor mask multiply; esr1_j scales the matmul weight columns
(g̃ = g*esr1, denominator column = esr1).

sl_i is needed for ALL i on every core; instead of an AllGather (which eats
~30us of inter-core skew + serialization before the main loop can start),
every core gets the full vert^T (f16) and a host-precomputed wa = W @ (0.8
a_l) [F,H], and projects sl = wa^T @ vert^T itself, chunked so it overlaps
the vert DMA stream.
"""

import sys

for _p in ("/opt/trn_rl_repo",):
    if _p not in sys.path:
        sys.path.append(_p)

import numpy as np
import ml_dtypes

import concourse.bass as bass
import concourse.bacc as bacc
import concourse.mybir as mybir
import concourse.tile as tile
from concourse.masks import make_identity

f32 = mybir.dt.float32
f16 = mybir.dt.float16

N = 4096          # nodes
F = 1433          # input features
FP = 1536         # padded features (12 * 128)
KT = FP // 128    # 12 contraction tiles
H = 8             # heads
DH = 8            # per-head dim
HD = H * DH       # 64
NC = 8            # cores
NL = N // NC      # 512 nodes per core
NCH = NL // 128   # 4 local j-chunks
NIS = N // 512    # 8 destination column slices
LRELU = 0.2

# (head, chunk) pairs whose max runs as relu+add on the ACT engine instead of
# one DVE tensor_scalar, to balance the two engines in the main loop.
ACT_MAX = {(h, 2) for h in range(H)} | {(h, 3) for h in range(0, H, 2)}

_STATE = {}

EXP = mybir.ActivationFunctionType.Exp
RELU = mybir.ActivationFunctionType.Relu
IDENT = mybir.ActivationFunctionType.Identity
COPY = mybir.ActivationFunctionType.Copy


def _build_program(repeat=1, null=False, nocc=False, debug=False, variant='b'):
    nc = bacc.Bacc("TRN2", target_bir_lowering=False, debug=False, num_devices=NC)

    vt = nc.dram_tensor("vt", [FP, NL], f16, kind="ExternalInput")
    vtf = nc.dram_tensor("vtf", [FP, N], f16, kind="ExternalInput")
    wp = nc.dram_tensor("wp", [FP, HD], f16, kind="ExternalInput")
    wa = nc.dram_tensor("wa", [FP, H], f16, kind="ExternalInput")
    ar = nc.dram_tensor("ar", [128, H], f32, kind="ExternalInput")
    mskt = nc.dram_tensor("mskt", [NL, N], f16, kind="ExternalInput")
    out = nc.dram_tensor("out", [NL, HD], f32, kind="ExternalOutput")

    # DRAM staging for the esl broadcast (SBUF sources can't be partition-
    # broadcast) + the two-stage ReduceScatter (heads 0-3 / heads 4-7)
    esl_dram = nc.dram_tensor("esl_dram", [H, N], f16)
    numtA = nc.dram_tensor("numtA", [N, 36], f32)
    numtB = nc.dram_tensor("numtB", [N, 36], f32)
    numt_rsA = nc.dram_tensor("numt_rsA", [NL, 36], f32)
    numt_rsB = nc.dram_tensor("numt_rsB", [NL, 36], f32)

    if null:
        with tile.TileContext(nc) as tc:
            with tc.tile_pool(name="np0", bufs=1) as p0:
                t0 = p0.tile([128, 64], f16)
                t1 = p0.tile([128, 64], f32)
                for b in range(NL // 128):
                    nc.sync.dma_start(t0[:], vt[128 * b:128 * (b + 1), 0:64])
                    nc.vector.tensor_copy(t1[:], t0[:])
                    nc.sync.dma_start(out[128 * b:128 * (b + 1), :], t1[:])
        nc.compile()
        return nc

    with tile.TileContext(nc) as tc:
        with (
            tc.tile_pool(name="const", bufs=1) as cp,
            tc.tile_pool(name="psum", bufs=8, space="PSUM") as pp,
        ):
            # ---- resident tiles ----
            w_sb = cp.tile([128, KT, HD], f16)
            nc.sync.dma_start(w_sb[:], wp[:].rearrange("(k p) d -> p k d", p=128))
            wa_sb = cp.tile([128, KT, H], f16)
            nc.sync.dma_start(wa_sb[:], wa[:].rearrange("(k p) d -> p k d", p=128))
            ar_sb = cp.tile([128, H], f32)
            nc.sync.dma_start(ar_sb[:], ar[:])
            msk_sb = cp.tile([128, NCH, N], f16)
            eslb_sb = cp.tile([128, H, N], f16)  # exp(0.8*sl_i) bcast, resident
            esl_sb = cp.tile([H, N], f16)        # exp(0.8*sl_i) row-major
            gt_sb = cp.tile([128, NL], f32)      # g^T padded to 128 partitions
            esr1_sb = cp.tile([128, NCH * H], f32)  # exp(sr), col 8c+h
            r_sb = cp.tile([128, NCH * H], f32)     # exp(-0.8 sr)
            nr_sb = cp.tile([128, NCH * H], f32)    # -exp(-0.8 sr) (ACT relu bias)
            gr_sb = cp.tile([128, NCH * 72], f16)  # lhsT per chunk/head + esr1 col
            ntb = cp.tile([128, N // 128, 72], f32, name="ntb")

            # ---- phase 1a: local projection (vt slice, single-pass f16) ----
            with tc.tile_pool(name="vtp", bufs=1) as vtp:
                vts = []
                for k in range(KT):
                    vtt = vtp.tile([128, NL], f16, name=f"vt{k}")
                    nc.sync.dma_start(vtt[:], vt[:].rearrange("(k p) n -> k p n", k=KT)[k])
                    vts.append(vtt)
                # mask chunk 0 early (first tt needs it right at phase-3 start)
                nc.sync.dma_start(
                    msk_sb[:, 0, :],
                    mskt[:].rearrange("(c p) i -> c p i", p=128)[0])

                nc.vector.memset(gt_sb[64:128, :], 0.0)
                gt_ps = pp.tile([128, 512], f32, tag="bank", name="gt_ps")
                for k in range(KT):
                    nc.tensor.matmul(gt_ps[0:HD, :], w_sb[:, k, :], vts[k][:],
                                     start=(k == 0), stop=(k == KT - 1))
                nc.vector.tensor_copy(gt_sb[0:HD, :], gt_ps[0:HD, :])

                for c in range(NCH):
                    # g for this chunk: [128 nodes, 64]
                    g_ps = pp.tile([128, 512], f32, tag="bank", name="g_ps")
                    for k in range(KT):
                        nc.tensor.matmul(g_ps[:, 0:HD], vts[k][:, 128 * c:128 * (c + 1)],
                                         w_sb[:, k, :], start=(k == 0),
                                         stop=(k == KT - 1))
                    # sr for this chunk
                    sr_ps = pp.tile([128, 512], f32, tag="bank", name="sr_ps")
                    nc.tensor.matmul(sr_ps[:, 0:H], gt_sb[:, 128 * c:128 * (c + 1)],
                                     ar_sb[:], start=True, stop=True)
                    nc.scalar.activation(esr1_sb[:, H * c:H * (c + 1)], sr_ps[:, 0:H],
                                         EXP)
                    nc.scalar.activation(r_sb[:, H * c:H * (c + 1)], sr_ps[:, 0:H],
                                         EXP, scale=-0.8)
                    nc.vector.tensor_scalar(nr_sb[:, H * c:H * (c + 1)],
                                            r_sb[:, H * c:H * (c + 1)], -1.0, None,
                                            mybir.AluOpType.mult)
                    # lhsT tile: [g_h*esr1 | esr1] interleaved, 9 cols per head
                    grc = gr_sb[:, 72 * c:72 * (c + 1)].rearrange("p (h k) -> p h k", k=9)
                    for h in range(H):
                        nc.vector.tensor_scalar(grc[:, h, 0:8], g_ps[:, 8 * h:8 * (h + 1)],
                                                esr1_sb[:, H * c + h:H * c + h + 1],
                                                None, mybir.AluOpType.mult)
                    nc.vector.tensor_copy(grc[:, :, 8], esr1_sb[:, H * c:H * (c + 1)])

                # ---- phase 1b: sl for ALL nodes from the full vert^T stream ----
                # 8 psum banks hold [8, 512] slices; vtf tiles stream through.
                sl_banks = [pp.tile([128, 512], f32, tag="bank", name=f"slb{s}")
                            for s in range(NIS)]
                with tc.tile_pool(name="vtfp", bufs=3) as vtfp:
                    for k in range(KT):
                        vft = vtfp.tile([128, N], f16, name="vtf")
                        nc.sync.dma_start(
                            vft[:], vtf[:].rearrange("(k p) n -> k p n", k=KT)[k])
                        for s in range(NIS):
                            nc.tensor.matmul(sl_banks[s][0:H, :], wa_sb[:, k, :],
                                             vft[:, 512 * s:512 * (s + 1)],
                                             start=(k == 0), stop=(k == KT - 1))
                for s in range(NIS):
                    nc.scalar.activation(esl_sb[:, 512 * s:512 * (s + 1)],
                                         sl_banks[s][0:H, :], EXP)
                nc.sync.dma_start(esl_dram[:], esl_sb[:])

            # ---- phase 2: broadcast esl rows to 128 partitions (via DRAM) ----
            # head 0 first (gates phase 3), then mask chunks 1-3, then the rest.
            def _bcast(h):
                _src = esl_dram[h:h + 1, :].rearrange("o (s f) -> o s f", s=NIS)
                nc.sync.dma_start(
                    eslb_sb[:, h, :].rearrange("p (s f) -> p s f", s=NIS),
                    _src.to_broadcast([128, NIS, NL]),
                )
            _bcast(0)
            for c in range(1, NCH):
                nc.sync.dma_start(
                    msk_sb[:, c, :],
                    mskt[:].rearrange("(c p) i -> c p i", p=128)[c])
            for h in range(1, H):
                _bcast(h)

            # ---- phase 3: main attention loop ----
            with (
                tc.tile_pool(name="tp", bufs=3) as tp,
                tc.tile_pool(name="urp", bufs=2) as urp,
                tc.tile_pool(name="pmp", bufs=4) as pmp,
                tc.tile_pool(name="small", bufs=4) as sp,
            ):
              for _rep in range(repeat):
                for h in range(H):
                    slb = eslb_sb[:, h, :]
                    # one PSUM bank holds the whole head: [128 dest, 32 blk * 9]
                    bank = pp.tile([128, 512], f32, tag="bank", name="bank")
                    for c in range(NCH):
                        u = tp.tile([128, N], f16, name="umax")
                        if (h, c) in ACT_MAX:
                            # max(esl, r) = r + relu(esl - r), on ACT
                            u1 = urp.tile([128, N], f16, name="urelu")
                            nc.scalar.activation(u1[:], slb,
                                                 RELU,
                                                 bias=nr_sb[:, H * c + h:H * c + h + 1])
                            nc.scalar.activation(u[:], u1[:],
                                                 IDENT,
                                                 bias=r_sb[:, H * c + h:H * c + h + 1])
                        else:
                            nc.vector.tensor_scalar(u[:], slb,
                                                    r_sb[:, H * c + h:H * c + h + 1],
                                                    None, mybir.AluOpType.max)
                        pm = pmp.tile([128, N], f16, name="pm")
                        nc.vector.tensor_tensor(pm[:], u[:], msk_sb[:, c, :],
                                                mybir.AluOpType.mult)
                        rhs = gr_sb[:, 72 * c + 9 * h:72 * c + 9 * (h + 1)]
                        for ib in range(N // 128):
                            nc.tensor.matmul(bank[:, 9 * ib:9 * (ib + 1)],
                                             pm[:, 128 * ib:128 * (ib + 1)], rhs,
                                             start=(c == 0 and ib == 0),
                                             stop=(c == NCH - 1 and ib == N // 128 - 1),
                                             skip_group_check=True)
                    # drain the head's bank on the ACT engine (DVE stays hot)
                    nc.scalar.activation(
                        ntb[:, :, 9 * h:9 * (h + 1)],
                        bank[:, 0:288].rearrange("p (b k) -> p b k", k=9), COPY)

                    if h == 3:
                        nc.sync.dma_start(
                            numtA[:].rearrange("(b p) m -> p b m", p=128),
                            ntb[:, :, 0:36])
                        if nocc:
                            nc.sync.dma_start(numt_rsA[:], numtA[0:NL, :])
                        else:
                            nc.gpsimd.collective_compute(
                                "ReduceScatter", mybir.AluOpType.add,
                                replica_groups=[list(range(NC))],
                                ins=[numtA[:].opt()], outs=[numt_rsA[:].opt()],
                            )
                if True:
                    nc.sync.dma_start(
                        numtB[:].rearrange("(b p) m -> p b m", p=128),
                        ntb[:, :, 36:72])
                    if nocc:
                        nc.sync.dma_start(numt_rsB[:], numtB[0:NL, :])
                    else:
                        nc.gpsimd.collective_compute(
                            "ReduceScatter", mybir.AluOpType.add,
                            replica_groups=[list(range(NC))],
                            ins=[numtB[:].opt()], outs=[numt_rsB[:].opt()],
                        )

                # ---- phase 6: divide + ELU, one pass per RS half ----
                for half, nsrc in ((0, numt_rsA), (1, numt_rsB)):
                    for b in range(NL // 128):
                        nf = sp.tile([128, 36], f32, name="nf")
                        nc.sync.dma_start(nf[:], nsrc[128 * b:128 * (b + 1), :])
                        nfr = nf.rearrange("p (h k) -> p h k", k=9)
                        rec = sp.tile([128, 4], f32, name="rec")
                        nc.vector.reciprocal(rec[:], nfr[:, :, 8])
                        aout = sp.tile([128, 32], f32, name="aout")
                        for hh in range(4):
                            nc.vector.tensor_scalar(aout[:, 8 * hh:8 * (hh + 1)],
                                                    nfr[:, hh, 0:8],
                                                    rec[:, hh:hh + 1], None,
                                                    mybir.AluOpType.mult)
                        # elu(x) = relu(x) - 1 + exp(min(x, 0))
                        xm = sp.tile([128, 32], f32, name="xm")
                        nc.vector.tensor_scalar(xm[:], aout[:], 0.0, None,
                                                mybir.AluOpType.min)
                        ex = sp.tile([128, 32], f32, name="ex")
                        nc.scalar.activation(ex[:], xm[:], EXP)
                        r1 = sp.tile([128, 32], f32, name="r1")
                        nc.vector.tensor_scalar(r1[:], aout[:], 0.0, -1.0,
                                                mybir.AluOpType.max,
                                                mybir.AluOpType.add)
                        ot = sp.tile([128, 32], f32, name="ot")
                        nc.vector.tensor_tensor(ot[:], ex[:], r1[:],
                                                mybir.AluOpType.add)
                        nc.sync.dma_start(
                            out[128 * b:128 * (b + 1), 32 * half:32 * (half + 1)],
                            ot[:])

    nc.compile()
    return nc


def _prep_inputs(vert, edge, W, a_l, a_r):
    vert = np.asarray(vert, dtype=np.float32)
    edge = np.asarray(edge)
    W = np.asarray(W, dtype=np.float32)
    a_l = np.asarray(a_l, dtype=np.float32)
    a_r = np.asarray(a_r, dtype=np.float32)

    vtp32 = np.zeros((FP, N), dtype=np.float32)
    vtp32[:F] = vert.T
    vtp = vtp32.astype(np.float16)
    wp32 = np.zeros((FP, HD), dtype=np.float32)
    wp32[:F] = W
    wp = wp32.astype(np.float16)

    # wa[f, h] = sum_d W[f, (h,d)] * 0.8 * a_l[h, d]
    wa = np.einsum('fhd,hd->fh', wp32.reshape(FP, H, DH), 0.8 * a_l)
    wa = wa.astype(np.float16)

    ar8 = np.zeros((128, H), dtype=np.float32)
    for h in range(H):
        ar8[8 * h:8 * (h + 1), h] = a_r[h]

    maskT = (edge != 0).astype(np.float16)  # [i, j] -> transpose below

    in_maps = []
    for c in range(NC):
        sl = slice(512 * c, 512 * (c + 1))
        in_maps.append({
            "vt": np.ascontiguousarray(vtp[:, sl]),
            "vtf": vtp,
            "wp": wp,
            "wa": wa,
            "ar": ar8,
            "mskt": np.ascontiguousarray(maskT[:, sl].T),
        })
    return in_maps


def _get_runner(repeat=1, null=False, variant='b'):
    """Build (once) and return a callable in_maps -> list of per-core outputs."""
    key = f"runner{repeat}_{null}_{variant}"
    if key in _STATE:
        return _STATE[key]

    nc = _build_program(repeat, null, variant=variant)
    _STATE[f"program{repeat}_{null}_{variant}"] = nc

    import jax
    from jax.sharding import Mesh, PartitionSpec
    from jax.experimental.shard_map import shard_map
    from concourse import bass2jax
    from concourse.bass2jax import _bass_exec_p, partition_id_tensor

    bass2jax.install_neuronx_cc_hook()

    partition_name = nc.partition_id_tensor.name if nc.partition_id_tensor else None
    in_names, out_names, out_avals, zero_shapes = [], [], [], []
    for alloc in nc.m.functions[0].allocations:
        if not isinstance(alloc, mybir.MemoryLocationSet):
            continue
        name = alloc.memorylocations[0].name
        if alloc.kind == "ExternalInput":
            if name != partition_name:
                in_names.append(name)
        elif alloc.kind == "ExternalOutput":
            shape = tuple(alloc.tensor_shape)
            dtype = mybir.dt.np(alloc.dtype)
            out_names.append(name)
            out_avals.append(jax.core.ShapedArray(shape, dtype))
            zero_shapes.append((shape, dtype))
    n_params = len(in_names)
    n_outs = len(out_avals)
    all_in_names = list(in_names) + list(out_names)
    if partition_name is not None:
        all_in_names.append(partition_name)
    donate = tuple(range(n_params, n_params + n_outs))

    def _body(*args):
        operands = list(args)
        if partition_name is not None:
            operands.append(partition_id_tensor())
        outs = _bass_exec_p.bind(
            *operands,
            out_avals=tuple(out_avals),
            in_names=tuple(all_in_names),
            out_names=tuple(out_names),
            lowering_input_output_aliases=(),
            sim_require_finite=True,
            sim_require_nnan=True,
            nc=nc,
        )
        return tuple(outs)

    devices = jax.devices()[:NC]
    mesh = Mesh(np.asarray(devices), ("core",))
    in_specs = (PartitionSpec("core"),) * (n_params + n_outs)
    out_specs = (PartitionSpec("core"),) * n_outs
    sharded = jax.jit(
        shard_map(_body, mesh=mesh, in_specs=in_specs, out_specs=out_specs,
                  check_rep=False),
        donate_argnums=donate, keep_unused=True,
    )

    def runner(in_maps):
        concat_in = [
            np.concatenate([np.asarray(in_maps[c][nm]) for c in range(NC)], axis=0)
            for nm in in_names
        ]
        concat_zeros = [
            np.zeros((NC * s[0], *s[1:]), dt) for (s, dt) in zero_shapes
        ]
        out_arrs = sharded(*concat_in, *concat_zeros)
        out_arrs = [np.asarray(a) for a in out_arrs]
        return [
            {nm: out_arrs[i].reshape(NC, *out_avals[i].shape)[c]
             for i, nm in enumerate(out_names)}
            for c in range(NC)
        ]

    _STATE[key] = runner
    _STATE[f"internals{repeat}_{null}_{variant}"] = {
        "sharded": sharded, "in_names": in_names, "zero_shapes": zero_shapes,
        "mesh": mesh, "out_names": out_names, "out_avals": out_avals,
    }
    return runner


def kernel(vert, edge, W, a_l, a_r):
    in_maps = _prep_inputs(vert, edge, W, a_l, a_r)
    runner = _get_runner()
    results = runner(in_maps)
    return np.concatenate([results[c]["out"] for c in range(NC)], axis=0)
